# revision 2
# baseline (speedup 1.0000x reference)
"""CLIF spiking-neuron recurrence kernel for 8 Trainium2 NeuronCores.

Reference semantics (per element, T=64 sequential steps, gamma=0.5):
    u     = 0.5*u + x_t
    spike = (u >= 1.0)
    m     = s_prev * sigmoid(0.5*u) + spike
    s     = sigmoid(m)                       # carried (in-place sigmoid_)
    u     = u - spike*(1.0 + s)
Output: spikes [T, B, D] float32.

v2 design (vs. the 5-hop baseline):
- Pure data parallel: 65536 elements/core as [128 x 512], G=2 column
  groups of 256 for pipelining.
- V = 2^t * u lives in ONE [128,512] PSUM bank (power-of-2 scaling is
  exact in fp32); leak folds into the ACT's free scale.
- The reset no longer needs sigmoid(m): 1+sigmoid(1+q) on q = s_prev*sg
  in [0,0.89] is approximated by a quadratic (max err 9.3e-5, validated
  offline: 18/33.5M spike flips vs reference, rel err 2.0e-3 << 2e-2).
  CLIF_Y3 (one DVE op, 2 tensor srcs) computes
      y = (sg >= c) * ((C2*q + C1)*q + 1)
  and the constant term's scale -2^t*b0 rides in per-step diagonal
  matmul weights, so the critical cycle is only 3 hops:
      ACT(sg) -> DVE(Y3) -> PE(V += W_t @ y) -> ACT(sg)
- M = s_prev*sg + spike and s = sigmoid(M) are computed OFF the cycle
  as single wide [128,512] ops (CLIF_M on DVE, sigmoid on ACT), feeding
  the next step's Y3/M with a full cycle of slack.
- Output: M itself. spike=1 <=> M >= 1 with huge margin (M <= 0.54 when
  spike=0), so M is DMA'd out as bf16 via the SWDGE cast path (gpsimd)
  - half the output bytes, zero compute-engine cost. Host compares >= 1.
- DMA is batched in 4-step/1MB chunks (in on the SP HWDGE ring, weights
  on the ACT ring, out on SWDGE) to amortize per-descriptor overhead.
"""

import sys
import types

import numpy as np
import ml_dtypes

# If BASS_TRACE is set but the image's antenv lacks axon_hooks,
# run_bass_kernel_spmd would crash importing it; install a null-hook
# module so tracing degrades gracefully instead.
try:
    import antenv.axon_hooks  # noqa: F401
except Exception:
    try:
        import antenv
        _hooks = types.ModuleType("antenv.axon_hooks")
        _hook_cell = [None]
        _hooks.set_axon_ntff_profile_hook = (
            lambda h: _hook_cell.__setitem__(0, h))
        _hooks.get_axon_ntff_profile_hook = lambda: _hook_cell[0]
        sys.modules["antenv.axon_hooks"] = _hooks
        antenv.axon_hooks = _hooks
    except Exception:
        pass

import concourse.bass as bass
import concourse.bacc as bacc
import concourse.mybir as mybir
import concourse.tile as tile
import concourse.dve_ops as dve_ops
from concourse.dve_spec import Spec, Src0, Src1, C0, C1, C2, One, lower, _has_src1
from concourse.dve_uop import DveOpSpec
from concourse.bass_utils import run_bass_kernel_spmd

F32 = mybir.dt.float32
BF16 = mybir.dt.bfloat16
AF = mybir.ActivationFunctionType

T = 64
B = 128
D = 4096
N_CORES = 8
P = 128
NPC = B * D // N_CORES          # 65536 elements per core
FDT = NPC // P                  # 512 free columns per core
CHUNK = 4                       # steps per input/output DMA chunk
NCHUNK = T // CHUNK

GROUPS = [(0, 256), (256, 256)]

# Quadratic fit of f(q) = 1 + sigmoid(1 + q) on q in [0, 0.9]
# (least-squares on 20001 points; max err 9.28e-5).  y3 computes
# (C2*q + C1)*q + 1 and the reset matmul weights carry -2^t * B0.
_B2 = -0.04712587
_B1 = 0.2907875 + 2.0 * _B2     # p(M)=c2 M^2+c1 M+c0 re-centered to q=M-1
_B0 = 1.48742689 + 0.2907875 + _B2
YC2 = float(np.float32(_B2 / _B0))
YC1 = float(np.float32(_B1 / _B0))

_NC_CACHE = None
LAST_RESULTS = None


def _register_dve_op(name, spec):
    for op in dve_ops.OPS:
        if op.name == name:
            return op
    shas = {}
    for ver in ("v3", "v4"):
        u = lower(spec, ver=ver)
        shas[ver] = DveOpSpec(name=name, opcode=1, uops=u,
                              rd1_en=_has_src1(spec)).sha(ver)
    op = dve_ops.DveOp(name, spec, subdim=False, uops_sha=shas)
    dve_ops.OPS.append(op)
    dve_ops._SUB_OPCODE_FOR_NAME[name] = (
        dve_ops._CUSTOM_DVE_ROW_BASE + len(dve_ops.OPS) - 1)
    dve_ops.CUSTOM_DVE_SPECS[name] = spec
    return op


# M = s_prev*sg + (sg >= c)          in0=s_prev, in1=sg, s0=c
CLIF_M = _register_dve_op("CLIF_M_ANT", Spec(
    body=Src0 * Src1 + (Src1 >= C0),
    reference=lambda in0, in1, s0, s1, imm2:
        in0 * in1 + (in1 >= s0).astype(np.float32),
))

# y = (sg >= c) * ((C2*q + C1)*q + 1),  q = s_prev*sg
#     in0=s_prev, in1=sg, s0=c, s1=C1, imm2=C2
_q = Src0 * Src1
CLIF_Y3 = _register_dve_op("CLIF_Y3_ANT", Spec(
    body=(Src1 >= C0) * ((C2 * _q + C1) * _q + One),
    reference=lambda in0, in1, s0, s1, imm2:
        (in1 >= s0).astype(np.float32)
        * ((imm2 * (in0 * in1) + s1) * (in0 * in1) + 1.0),
))


def _build():
    nc = bacc.Bacc(None, target_bir_lowering=False, debug=False,
                   num_devices=N_CORES)

    xs = nc.declare_dram_parameter("xs", [NCHUNK, P, CHUNK, FDT], F32,
                                   isOutput=False)
    wt = nc.declare_dram_parameter("wt", [P, P], F32, isOutput=False)  # identity
    wts = nc.declare_dram_parameter("wts", [P, T, P], F32,
                                    isOutput=False)  # -2^t*B0 diagonals
    out = nc.declare_dram_parameter("out", [NCHUNK, P, CHUNK, FDT], BF16,
                                    isOutput=True)

    with tile.TileContext(nc) as tc:
        with (
            tc.tile_pool(name="wpool", bufs=1) as wpool,
            tc.tile_pool(name="cpool", bufs=1) as cpool,
            tc.tile_pool(name="xpool", bufs=2) as xpool,
            tc.tile_pool(name="mpool", bufs=2) as mpool,
            tc.tile_pool(name="sgpool", bufs=3) as sgpool,
            tc.tile_pool(name="spool", bufs=3) as spool,
            tc.tile_pool(name="ypool", bufs=4) as ypool,
            tc.tile_pool(name="vpool", bufs=1, space="PSUM") as vpool,
        ):
            # --- one-time setup -------------------------------------------
            eye = wpool.tile([P, P], F32, tag="eye")
            nc.sync.dma_start(eye[:], wt[:])
            wtile = wpool.tile([P, T, P], F32, tag="wts")
            # reset weights: first 8 steps' tiles first so step 0 isn't
            # gated on the full 4MB transfer; both ride the ACT HWDGE ring
            nc.scalar.dma_start(wtile[:, 0:8, :], wts[:, 0:8, :])
            nc.scalar.dma_start(wtile[:, 8:T, :], wts[:, 8:T, :])

            halft = cpool.tile([P, 1], F32, tag="half")
            nc.gpsimd.memset(halft[:], 0.5)
            ct = cpool.tile([P, 1], F32, tag="c")
            # c = sigmoid_LUT(0.5), same LUT as the per-step sigmoids
            nc.scalar.activation(ct[:], halft[:], AF.Sigmoid, bias=0.0, scale=1.0)
            c_ap = ct[:, 0:1]

            # --- initial state --------------------------------------------
            sw_prev = spool.tile([P, FDT], F32, tag="sw")
            nc.gpsimd.memset(sw_prev[:], 0.0)

            V = vpool.tile([P, FDT], F32, tag="V")

            # PE warm-up: dummy matmuls fill the otherwise-idle prologue
            # window so the HAM clock gate reaches 2.4 GHz before the first
            # real matmul
            junk = vpool.tile([P, 128], F32, tag="junk")
            for _ in range(10):
                nc.tensor.matmul(junk[:], eye[:], eye[:], start=True, stop=True)

            cur_x = xpool.tile([P, CHUNK, FDT], F32, tag="x")
            nc.sync.dma_start(cur_x[:], xs[0])
            nxt_x = None

            nc.tensor.matmul(V[:], eye[:], cur_x[:, 0, :],
                             start=True, stop=False, skip_group_check=True)

            # --- the recurrence -------------------------------------------
            mb = None
            for t in range(T):
                ci = t % CHUNK
                if ci == 0:
                    mb = mpool.tile([P, CHUNK, FDT], F32, tag="m")
                    if t + CHUNK < T:
                        nxt_x = xpool.tile([P, CHUNK, FDT], F32, tag="x")
                        nc.sync.dma_start(nxt_x[:], xs[(t + CHUNK) // CHUNK])

                sc_sg = float(2.0 ** (-t - 1))

                # critical cycle: sg -> y3 -> reset matmul
                sgw = sgpool.tile([P, FDT], F32, tag="sg")
                for o, w in GROUPS:
                    nc.scalar.activation(sgw[:, o:o + w], V[:, o:o + w],
                                         AF.Sigmoid, bias=0.0, scale=sc_sg)

                if t < T - 1:
                    # input add for the NEXT step: off the critical loop,
                    # emitted early so the PE runs it while DVE does y3
                    xsrc = nxt_x if ci == CHUNK - 1 else cur_x
                    nc.tensor.matmul(V[:], eye[:], xsrc[:, (ci + 1) % CHUNK, :],
                                     start=False, stop=False,
                                     skip_group_check=True)

                    ys = []
                    for o, w in GROUPS:
                        y = ypool.tile([P, w], F32, tag=f"y{o}")
                        nc.vector._custom_dve(CLIF_Y3, out=y[:],
                                              in0=sw_prev[:, o:o + w],
                                              in1=sgw[:, o:o + w],
                                              s0=c_ap, s1=YC1, imm2=YC2)
                        ys.append(y)
                    for gi, (o, w) in enumerate(GROUPS):
                        nc.tensor.matmul(V[:, o:o + w], wtile[:, t, :],
                                         ys[gi][:],
                                         start=False,
                                         stop=(t == T - 2 and gi == len(GROUPS) - 1),
                                         skip_group_check=True)

                # off-cycle: M (the output; also feeds s) and s = sigmoid(M)
                nc.vector._custom_dve(CLIF_M, out=mb[:, ci, :],
                                      in0=sw_prev[:], in1=sgw[:], s0=c_ap)
                if t < T - 1:
                    sw_new = spool.tile([P, FDT], F32, tag="sw")
                    nc.scalar.activation(sw_new[:], mb[:, ci, :], AF.Sigmoid,
                                         bias=0.0, scale=1.0)
                    sw_prev = sw_new

                if ci == CHUNK - 1:
                    # bf16 cast-on-DMA (SWDGE): half the output bytes
                    nc.gpsimd.dma_start(out[t // CHUNK], mb[:])
                    if t < T - 1:
                        cur_x = nxt_x

    nc.compile()
    return nc


def _get_nc():
    global _NC_CACHE
    if _NC_CACHE is None:
        _NC_CACHE = _build()
    return _NC_CACHE


def kernel(x_seq: np.ndarray) -> np.ndarray:
    global LAST_RESULTS
    x = np.ascontiguousarray(x_seq, dtype=np.float32)
    assert x.shape == (T, B, D), x.shape

    # 2^t prescale (exact in fp32), per-core shard, chunk-major layout
    scale = (2.0 ** np.arange(T, dtype=np.float64)).astype(np.float32)
    xsc = x.reshape(T, -1) * scale[:, None]
    xsc = xsc.reshape(T, N_CORES, P, FDT)

    eye_host = np.eye(P, dtype=np.float32)
    w_host = np.zeros((P, T, P), dtype=np.float32)
    diag_vals = (-(2.0 ** np.arange(T, dtype=np.float64)) * _B0).astype(np.float32)
    pi = np.arange(P)
    w_host[pi[:, None], np.arange(T)[None, :], pi[:, None]] = diag_vals[None, :]

    nc = _get_nc()
    in_maps = []
    for c in range(N_CORES):
        xc = xsc[:, c].reshape(NCHUNK, CHUNK, P, FDT).transpose(0, 2, 1, 3)
        in_maps.append({
            "xs": np.ascontiguousarray(xc),
            "wt": eye_host,
            "wts": w_host,
        })
    LAST_RESULTS = run_bass_kernel_spmd(nc, in_maps, list(range(N_CORES)))

    full = np.empty((T, N_CORES, P, FDT), dtype=np.float32)
    for c in range(N_CORES):
        res = LAST_RESULTS.results[c]
        m = np.asarray(res["out"]).astype(np.float32)       # [NCHUNK,P,CHUNK,FDT]
        m = m.transpose(0, 2, 1, 3).reshape(T, P, FDT)
        full[:, c] = (m >= 1.0).astype(np.float32)
    return full.reshape(T, B, D)


# revision 6
# speedup vs baseline: 1.0670x; 1.0670x over previous
"""CLIF spiking-neuron recurrence kernel for 8 Trainium2 NeuronCores.

Reference semantics (per element, T=64 sequential steps, gamma=0.5):
    u     = 0.5*u + x_t
    spike = (u >= 1.0)
    m     = s_prev * sigmoid(0.5*u) + spike
    s     = sigmoid(m)                       # carried (in-place sigmoid_)
    u     = u - spike*(1.0 + s)
Output: spikes [T, B, D] float32.

v3 design:
- Pure data parallel: 65536 elements/core as [128 x 512], G=2 column
  groups of 256 for pipelining; V = 2^t * u in ONE [128,512] PSUM bank
  (power-of-2 scaling exact in fp32; leak folds into ACT's free scale).
- The reset avoids sigmoid(m): 1+sigmoid(1+q), q = s_prev*sg, is a
  quadratic with the constant term CONSTRAINED to the bf16-representable
  b0'=1.734375 (fit on q in [0.30,0.93] - for t>=1 spiking elements
  always have q >= 0.311 since s_prev>=0.5, sg>=c). t=0 has q == 0
  exactly and uses a one-off fp32 path with the exact constant.
  Offline fp32 simulation of this exact arithmetic (incl. fp16/bf16
  rounding): 454/33.5M spike flips, rel err 1.02e-2 < 2e-2. Prior sims
  predicted HW flip counts exactly (1/1, 18/18).
- Critical cycle is 3 hops: ACT(sg) -> DVE(Y3) -> PE(V += W_t @ y).
  CLIF_Y3 emits y = (sg>=c)*((C2*q+C1)*q+1) in fp16; the reset matmul
  is bf16 stationary (-2^t*b0' exact diagonals) x fp16 moving = 1-pass,
  exact products in fp32 PSUM accum.
- Input adds are bf16 hi+lo split matmuls (exact to 2^-17 of x), issued
  TWO steps ahead so the PE's weight self-loads never block the cycle.
- M = s_prev*sg + spike (wide DVE) and s = sigmoid(M) (wide ACT) run off
  the cycle with a full step of slack.
- Output: M as bf16 via the SWDGE cast DMA path (spike=1 <=> M>=1 with
  margin 0.55 vs 1.0, preserved under any rounding). Host compares >= 1.
- DMA batched in 4-step chunks; inputs ride the SP HWDGE ring, weights
  the ACT ring, output casts on SWDGE.
"""

import sys
import types

import numpy as np
import ml_dtypes

# If BASS_TRACE is set but the image's antenv lacks axon_hooks,
# run_bass_kernel_spmd would crash importing it; install a null-hook
# module so tracing degrades gracefully instead.
try:
    import antenv.axon_hooks  # noqa: F401
except Exception:
    try:
        import antenv
        _hooks = types.ModuleType("antenv.axon_hooks")
        _hook_cell = [None]
        _hooks.set_axon_ntff_profile_hook = (
            lambda h: _hook_cell.__setitem__(0, h))
        _hooks.get_axon_ntff_profile_hook = lambda: _hook_cell[0]
        sys.modules["antenv.axon_hooks"] = _hooks
        antenv.axon_hooks = _hooks
    except Exception:
        pass

import concourse.bass as bass
import concourse.bacc as bacc
import concourse.mybir as mybir
import concourse.tile as tile
import concourse.dve_ops as dve_ops
from concourse.dve_spec import Spec, Src0, Src1, C0, C1, C2, One, lower, _has_src1
from concourse.dve_uop import DveOpSpec
from concourse.bass_utils import run_bass_kernel_spmd

F32 = mybir.dt.float32
BF16 = mybir.dt.bfloat16
FP16 = mybir.dt.float16
AF = mybir.ActivationFunctionType

T = 64
B = 128
D = 4096
N_CORES = 8
P = 128
NPC = B * D // N_CORES          # 65536 elements per core
FDT = NPC // P                  # 512 free columns per core
CHUNK = 4                       # steps per input/output DMA chunk
NCHUNK = T // CHUNK

GROUPS = [(0, 256), (256, 256)]

# Constrained LS fit of f(q) = 1 + sigmoid(1 + q) on q in [0.30, 0.93]
# with constant term forced to the bf16-exact B0P (max err 6.9e-4; only
# reachable q values matter - see module docstring).
B0P = 1.734375                          # bf16-exact
B1C = 0.18530899
B2C = -0.03826911
YC1 = float(np.float32(B1C / B0P))
YC2 = float(np.float32(B2C / B0P))
B0_EXACT = float(np.float32(1.0 + 1.0 / (1.0 + np.exp(-1.0))))

_NC_CACHE = None
LAST_RESULTS = None


def _register_dve_op(name, spec):
    for op in dve_ops.OPS:
        if op.name == name:
            return op
    shas = {}
    for ver in ("v3", "v4"):
        u = lower(spec, ver=ver)
        shas[ver] = DveOpSpec(name=name, opcode=1, uops=u,
                              rd1_en=_has_src1(spec)).sha(ver)
    op = dve_ops.DveOp(name, spec, subdim=False, uops_sha=shas)
    dve_ops.OPS.append(op)
    dve_ops._SUB_OPCODE_FOR_NAME[name] = (
        dve_ops._CUSTOM_DVE_ROW_BASE + len(dve_ops.OPS) - 1)
    dve_ops.CUSTOM_DVE_SPECS[name] = spec
    return op


# M = s_prev*sg + (sg >= c)          in0=s_prev, in1=sg, s0=c
CLIF_M = _register_dve_op("CLIF_M_ANT", Spec(
    body=Src0 * Src1 + (Src1 >= C0),
    reference=lambda in0, in1, s0, s1, imm2:
        in0 * in1 + (in1 >= s0).astype(np.float32),
))

# y = (sg >= c) * ((C2*q + C1)*q + 1),  q = s_prev*sg
#     in0=s_prev, in1=sg, s0=c, s1=C1, imm2=C2
_q = Src0 * Src1
CLIF_Y3 = _register_dve_op("CLIF_Y3_ANT", Spec(
    body=(Src1 >= C0) * ((C2 * _q + C1) * _q + One),
    reference=lambda in0, in1, s0, s1, imm2:
        (in1 >= s0).astype(np.float32)
        * ((imm2 * (in0 * in1) + s1) * (in0 * in1) + 1.0),
))


def _build():
    nc = bacc.Bacc(None, target_bir_lowering=False, debug=False,
                   num_devices=N_CORES)

    xsh = nc.declare_dram_parameter("xsh", [NCHUNK, P, CHUNK, FDT], BF16,
                                    isOutput=False)
    xsl = nc.declare_dram_parameter("xsl", [NCHUNK, P, CHUNK, FDT], BF16,
                                    isOutput=False)
    wt = nc.declare_dram_parameter("wt", [P, P], BF16, isOutput=False)   # identity
    w0 = nc.declare_dram_parameter("w0", [P, P], F32, isOutput=False)    # -B0*I
    wts = nc.declare_dram_parameter("wts", [P, T, P], BF16,
                                    isOutput=False)  # -2^t*B0P diagonals
    out = nc.declare_dram_parameter("out", [NCHUNK, P, CHUNK, FDT], BF16,
                                    isOutput=True)

    with tile.TileContext(nc) as tc:
        with (
            tc.tile_pool(name="wpool", bufs=1) as wpool,
            tc.tile_pool(name="cpool", bufs=1) as cpool,
            tc.tile_pool(name="xpool", bufs=2) as xpool,
            tc.tile_pool(name="mpool", bufs=2) as mpool,
            tc.tile_pool(name="sgpool", bufs=3) as sgpool,
            tc.tile_pool(name="spool", bufs=3) as spool,
            tc.tile_pool(name="ypool", bufs=4) as ypool,
            tc.tile_pool(name="vpool", bufs=1, space="PSUM") as vpool,
        ):
            # --- one-time setup -------------------------------------------
            eyeb = wpool.tile([P, P], BF16, tag="eye")
            nc.sync.dma_start(eyeb[:], wt[:])
            w0t = wpool.tile([P, P], F32, tag="w0")
            nc.scalar.dma_start(w0t[:], w0[:])
            wtile = wpool.tile([P, T, P], BF16, tag="wts")
            nc.scalar.dma_start(wtile[:, 0:8, :], wts[:, 0:8, :])
            nc.scalar.dma_start(wtile[:, 8:T, :], wts[:, 8:T, :])

            halft = cpool.tile([P, 1], F32, tag="half")
            nc.gpsimd.memset(halft[:], 0.5)
            ct = cpool.tile([P, 1], F32, tag="c")
            # c = sigmoid_LUT(0.5), same LUT as the per-step sigmoids
            nc.scalar.activation(ct[:], halft[:], AF.Sigmoid, bias=0.0, scale=1.0)
            c_ap = ct[:, 0:1]

            # --- initial state --------------------------------------------
            sw_prev = spool.tile([P, FDT], F32, tag="sw")
            nc.gpsimd.memset(sw_prev[:], 0.0)

            V = vpool.tile([P, FDT], F32, tag="V")

            # PE warm-up: dummy matmuls so the HAM clock gate reaches
            # 2.4 GHz before the first real matmul
            junk = vpool.tile([P, 128], F32, tag="junk")
            for _ in range(10):
                nc.tensor.matmul(junk[:], eyeb[:], eyeb[:], start=True, stop=True)

            xtiles = {}
            xh0 = xpool.tile([P, CHUNK, FDT], BF16, tag="xh")
            xl0 = xpool.tile([P, CHUNK, FDT], BF16, tag="xl")
            nc.sync.dma_start(xh0[:], xsh[0])
            nc.sync.dma_start(xl0[:], xsl[0])
            xtiles[0] = (xh0, xl0)

            # input for step 0 (in-loop adds run one step ahead)
            nc.tensor.matmul(V[:], eyeb[:], xh0[:, 0, :],
                             start=True, stop=False, skip_group_check=True)
            nc.tensor.matmul(V[:], eyeb[:], xl0[:, 0, :],
                             start=False, stop=False, skip_group_check=True)

            # --- the recurrence -------------------------------------------
            mb = None
            for t in range(T):
                ci = t % CHUNK
                k = t // CHUNK
                if ci == 0:
                    mb = mpool.tile([P, CHUNK, FDT], F32, tag="m")
                    if k + 1 < NCHUNK:
                        xh = xpool.tile([P, CHUNK, FDT], BF16, tag="xh")
                        xl = xpool.tile([P, CHUNK, FDT], BF16, tag="xl")
                        nc.sync.dma_start(xh[:], xsh[k + 1])
                        nc.sync.dma_start(xl[:], xsl[k + 1])
                        xtiles[k + 1] = (xh, xl)
                        xtiles.pop(k - 1, None)

                sc_sg = float(2.0 ** (-t - 1))

                # critical cycle: sg -> y3 -> reset matmul
                sgw = sgpool.tile([P, FDT], F32, tag="sg")
                for o, w in GROUPS:
                    nc.scalar.activation(sgw[:, o:o + w], V[:, o:o + w],
                                         AF.Sigmoid, bias=0.0, scale=sc_sg)

                # input adds for step t+1: emitted right after the sg reads
                # of V (Tile orders the WAR edge correctly) and executed on
                # the PE while the DVE runs Y3, so they stay off the cycle
                if t + 1 < T:
                    t1 = t + 1
                    xh, xl = xtiles[t1 // CHUNK]
                    nc.tensor.matmul(V[:], eyeb[:], xh[:, t1 % CHUNK, :],
                                     start=False, stop=False,
                                     skip_group_check=True)
                    nc.tensor.matmul(V[:], eyeb[:], xl[:, t1 % CHUNK, :],
                                     start=False, stop=False,
                                     skip_group_check=True)

                if t < T - 1:
                    ydt = F32 if t == 0 else FP16
                    ys = []
                    for o, w in GROUPS:
                        y = ypool.tile([P, w], ydt, tag=f"y{o}")
                        nc.vector._custom_dve(CLIF_Y3, out=y[:],
                                              in0=sw_prev[:, o:o + w],
                                              in1=sgw[:, o:o + w],
                                              s0=c_ap, s1=YC1, imm2=YC2)
                        ys.append(y)
                    wsrc = w0t[:] if t == 0 else wtile[:, t, :]
                    for gi, (o, w) in enumerate(GROUPS):
                        nc.tensor.matmul(V[:, o:o + w], wsrc, ys[gi][:],
                                         start=False,
                                         stop=(t == T - 2 and gi == len(GROUPS) - 1),
                                         skip_group_check=True)

                # off-cycle: M (the output; also feeds s) and s = sigmoid(M)
                nc.vector._custom_dve(CLIF_M, out=mb[:, ci, :],
                                      in0=sw_prev[:], in1=sgw[:], s0=c_ap)
                if t < T - 1:
                    sw_new = spool.tile([P, FDT], F32, tag="sw")
                    nc.scalar.activation(sw_new[:], mb[:, ci, :], AF.Sigmoid,
                                         bias=0.0, scale=1.0)
                    sw_prev = sw_new

                if ci == CHUNK - 1:
                    # bf16 cast-on-DMA (SWDGE): half the output bytes
                    nc.gpsimd.dma_start(out[k], mb[:])

    nc.compile()
    return nc


def _get_nc():
    global _NC_CACHE
    if _NC_CACHE is None:
        _NC_CACHE = _build()
    return _NC_CACHE


def kernel(x_seq: np.ndarray) -> np.ndarray:
    global LAST_RESULTS
    x = np.ascontiguousarray(x_seq, dtype=np.float32)
    assert x.shape == (T, B, D), x.shape

    # 2^t prescale (exact in fp32), bf16 hi/lo split, per-core shard,
    # chunk-major layout
    scale = 2.0 ** np.arange(T, dtype=np.float64)
    xsc = (x.reshape(T, -1).astype(np.float64) * scale[:, None]).astype(np.float32)
    xh = xsc.astype(ml_dtypes.bfloat16)
    xl = (xsc - xh.astype(np.float32)).astype(ml_dtypes.bfloat16)
    xh = xh.reshape(T, N_CORES, P, FDT)
    xl = xl.reshape(T, N_CORES, P, FDT)

    eye_host = np.eye(P, dtype=np.float32).astype(ml_dtypes.bfloat16)
    w0_host = (-B0_EXACT * np.eye(P, dtype=np.float32)).astype(np.float32)
    w_host = np.zeros((P, T, P), dtype=np.float32)
    diag_vals = (-(2.0 ** np.arange(T, dtype=np.float64)) * B0P).astype(np.float32)
    pi = np.arange(P)
    w_host[pi[:, None], np.arange(T)[None, :], pi[:, None]] = diag_vals[None, :]
    w_host = w_host.astype(ml_dtypes.bfloat16)

    nc = _get_nc()
    in_maps = []
    for c in range(N_CORES):
        xhc = xh[:, c].reshape(NCHUNK, CHUNK, P, FDT).transpose(0, 2, 1, 3)
        xlc = xl[:, c].reshape(NCHUNK, CHUNK, P, FDT).transpose(0, 2, 1, 3)
        in_maps.append({
            "xsh": np.ascontiguousarray(xhc),
            "xsl": np.ascontiguousarray(xlc),
            "wt": eye_host,
            "w0": w0_host,
            "wts": w_host,
        })
    LAST_RESULTS = run_bass_kernel_spmd(nc, in_maps, list(range(N_CORES)))

    full = np.empty((T, N_CORES, P, FDT), dtype=np.float32)
    for c in range(N_CORES):
        res = LAST_RESULTS.results[c]
        m = np.asarray(res["out"]).astype(np.float32)       # [NCHUNK,P,CHUNK,FDT]
        m = m.transpose(0, 2, 1, 3).reshape(T, P, FDT)
        full[:, c] = (m >= 1.0).astype(np.float32)
    return full.reshape(T, B, D)
